# revision 6
# baseline (speedup 1.0000x reference)
"""DePatchEmbed Trainium2 kernel.

Full op: x (32, 16384, 256) f32 -> out (32, 64, 256, 256) f32 with
  out[n, c, 2*ih+pi, 2*jw+pj] = x[n, jw*128+ih, c*4+pi*2+pj]

Sharding: pure data-parallel over the batch dim — 4 examples per core on
8 NeuronCores. Per core the op is a local permutation done in one pass:

  load : x[n] -> L[ih; jw, d]  SBUF, partition = ih (1 KiB contiguous runs)
  DVE  : S[ih; cl, pi, w=2jw+pj] <- L[ih; jw, (c,pi,pj)]  (strided copies,
         data never leaves its partition)
  store: S -> out[n, c-block]  (2 KiB contiguous runs)
"""

import json

import numpy as np

import concourse.bass as bass
import concourse.bass_utils
import concourse.bass2jax
import concourse.mybir as mybir
from concourse import tile
from concourse.bass_utils import run_bass_kernel_spmd

F32 = mybir.dt.float32

# ---------------------------------------------------------------------------
# The bundled walrus accepts at most one sync-wait per instruction
# ("Too many sync wait commands" in CoreV3GenImpl::setupSyncWait), but Tile's
# kernel-tail Drain carries one wait per outstanding DMA-sem lane. Rewrite the
# BIR before compilation: split any instruction with N>1 waits into N-1
# single-wait Drains followed by the original instruction with one wait.
_ORIG_COMPILE_BIR = concourse.bass_utils.compile_bir_kernel


def _split_multiwait_bir(bir_json: bytes) -> bytes:
    bir = json.loads(bir_json)
    changed = False
    first_block = True
    for fn in bir.get("functions", []):
        for bb in fn.get("blocks", []):
            insts = bb.get("instructions", [])
            out = []
            for inst in insts:
                # Strip the entry const-pool barrier (this kernel reads no
                # const APs, so engines need not rendezvous before starting).
                if first_block and inst.get("opcode") in ("Drain", "EventSemaphore"):
                    si0 = inst.get("sync_info") or {}
                    sems = [
                        w.get("ant_name", "")
                        for w in si0.get("on_wait", []) + si0.get("on_update", [])
                    ]
                    if sems and all(s.startswith("barrier_") for s in sems):
                        changed = True
                        continue
                si = inst.get("sync_info")
                waits = si.get("on_wait", []) if si else []
                if len(waits) > 1:
                    changed = True
                    for k, w in enumerate(waits[:-1]):
                        out.append(
                            {
                                "debug": inst.get("debug", 0),
                                "engine": inst["engine"],
                                "ins": [],
                                "outs": [],
                                "is_reset_sema": False,
                                "name": f"{inst['name']}-sw{k}",
                                "opcode": "Drain",
                                "sync_info": {"on_update": [], "on_wait": [w]},
                            }
                        )
                    si["on_wait"] = [waits[-1]]
                out.append(inst)
            bb["instructions"] = out
            first_block = False
    if not changed:
        return bir_json
    return json.dumps(bir).encode()


def _patched_compile_bir_kernel(bir_json, tmpdir, neff_name="file.neff"):
    return _ORIG_COMPILE_BIR(_split_multiwait_bir(bir_json), tmpdir, neff_name)


if getattr(concourse.bass2jax.compile_bir_kernel, "__name__", "") != (
    "_patched_compile_bir_kernel"
):
    concourse.bass2jax.compile_bir_kernel = _patched_compile_bir_kernel
    concourse.bass_utils.compile_bir_kernel = _patched_compile_bir_kernel

N_CORES = 8
N_FULL = 32     # full batch
NB = N_FULL // N_CORES  # examples per core
HG = 128        # H // P
WG = 128        # W // P
C = 64          # channels
P = 2           # patch size
DIM = C * P * P             # 256 floats per patch row
LFREE = WG * DIM            # floats per partition for one example
CB = 8                      # channels per store block
NCB = C // CB
SFREE = CB * P * 256
NJB = 8                     # load chunks per example
JB = WG // NJB


def _build_kernel(nc: bass.Bass, x: bass.AP, out: bass.AP):
    with tile.TileContext(nc) as tc:
        with (
            tc.tile_pool(name="lpool", bufs=1) as lpool,
            tc.tile_pool(name="spool", bufs=3) as spool,
        ):
            for n in range(NB):
                xv = x[n].rearrange("(jw ih) d -> ih jw d", ih=HG)
                ov = out[n].rearrange("c (ih pi) w -> ih c (pi w)", ih=HG)
                L = lpool.tile([128, LFREE], F32, tag="L")
                lv = L.rearrange("p (jw d) -> p jw d", d=DIM)
                # two big load DMAs (jw halves) to minimize inter-inst stalls
                JH = WG // 2
                for h in range(2):
                    nc.sync.dma_start(
                        out=lv[:, h * JH : (h + 1) * JH, :],
                        in_=xv[:, h * JH : (h + 1) * JH, :],
                    )
                lshuf = L.rearrange(
                    "p (jw c pi pj) -> p jw c pi pj", jw=WG, c=C, pi=P, pj=P
                )

                def copy_block(cb, pi, pj, j0, j1, sv):
                    src = lshuf[:, j0:j1, cb * CB : (cb + 1) * CB, pi, pj]
                    src = src.transpose([0, 2, 1])  # [p, cl, jw-range]
                    dst = sv[:, :, pi, j0:j1, pj]   # [p, cl, jw-range]
                    nc.vector.tensor_copy(out=dst, in_=src)

                for cb in range(NCB):
                    S = spool.tile([128, SFREE], F32, tag="S")
                    sv = S.rearrange(
                        "p (cl pi jw pj) -> p cl pi jw pj", cl=CB, pi=P, jw=WG, pj=P
                    )
                    if cb == 0:
                        # split jw so the first half's copies overlap the
                        # second load DMA, letting this store start ~1us
                        # after the load phase ends
                        for pi in range(P):
                            for pj in range(P):
                                copy_block(cb, pi, pj, 0, JH, sv)
                        for pi in range(P):
                            for pj in range(P):
                                copy_block(cb, pi, pj, JH, WG, sv)
                    else:
                        for pi in range(P):
                            for pj in range(P):
                                copy_block(cb, pi, pj, 0, WG, sv)
                    nc.scalar.dma_start(
                        out=ov[:, cb * CB : (cb + 1) * CB, :],
                        in_=S.rearrange("p (cl piw) -> p cl piw", piw=P * 256),
                    )


_NC_CACHE = None


def _get_program() -> bass.Bass:
    global _NC_CACHE
    if _NC_CACHE is None:
        nc = bass.Bass("TRN2", num_devices=N_CORES)
        x = nc.dram_tensor("x", [NB, WG * HG, DIM], F32, kind="ExternalInput")
        out = nc.dram_tensor(
            "out", [NB, C, HG * P, WG * P], F32, kind="ExternalOutput"
        )
        _build_kernel(nc, x.ap(), out.ap())
        _NC_CACHE = nc
    return _NC_CACHE


def kernel(x: np.ndarray, H=256, W=256, **_unused) -> np.ndarray:
    x = np.ascontiguousarray(x, dtype=np.float32)
    assert x.shape == (N_FULL, WG * HG, DIM), x.shape
    nc = _get_program()
    shards = np.split(x, N_CORES, axis=0)
    in_maps = [{"x": s} for s in shards]
    res = run_bass_kernel_spmd(nc, in_maps, core_ids=list(range(N_CORES)))
    outs = [np.asarray(r["out"]) for r in res.results]
    return np.concatenate(outs, axis=0)


# revision 7
# speedup vs baseline: 1.0163x; 1.0163x over previous
"""DePatchEmbed Trainium2 kernel.

Full op: x (32, 16384, 256) f32 -> out (32, 64, 256, 256) f32 with
  out[n, c, 2*ih+pi, 2*jw+pj] = x[n, jw*128+ih, c*4+pi*2+pj]

Sharding: pure data-parallel over the batch dim — 4 examples per core on
8 NeuronCores. Per core the op is a local permutation done in one pass:

  load : x[n] -> L[ih; jw, d]  SBUF, partition = ih (1 KiB contiguous runs)
  DVE  : S[ih; cl, pi, w=2jw+pj] <- L[ih; jw, (c,pi,pj)]  (strided copies,
         data never leaves its partition)
  store: S -> out[n, c-block]  (2 KiB contiguous runs)
"""

import json

import numpy as np

import concourse.bass as bass
import concourse.bass_utils
import concourse.bass2jax
import concourse.mybir as mybir
from concourse import tile
from concourse.bass_utils import run_bass_kernel_spmd

F32 = mybir.dt.float32

# ---------------------------------------------------------------------------
# The bundled walrus accepts at most one sync-wait per instruction
# ("Too many sync wait commands" in CoreV3GenImpl::setupSyncWait), but Tile's
# kernel-tail Drain carries one wait per outstanding DMA-sem lane. Rewrite the
# BIR before compilation: split any instruction with N>1 waits into N-1
# single-wait Drains followed by the original instruction with one wait.
_ORIG_COMPILE_BIR = concourse.bass_utils.compile_bir_kernel


def _split_multiwait_bir(bir_json: bytes) -> bytes:
    bir = json.loads(bir_json)
    changed = False
    first_block = True
    for fn in bir.get("functions", []):
        for bb in fn.get("blocks", []):
            insts = bb.get("instructions", [])
            out = []
            for inst in insts:
                # Strip the entry const-pool barrier (this kernel reads no
                # const APs, so engines need not rendezvous before starting).
                if first_block and inst.get("opcode") in ("Drain", "EventSemaphore"):
                    si0 = inst.get("sync_info") or {}
                    sems = [
                        w.get("ant_name", "")
                        for w in si0.get("on_wait", []) + si0.get("on_update", [])
                    ]
                    if sems and all(s.startswith("barrier_") for s in sems):
                        changed = True
                        continue
                si = inst.get("sync_info")
                waits = si.get("on_wait", []) if si else []
                if len(waits) > 1:
                    changed = True
                    for k, w in enumerate(waits[:-1]):
                        out.append(
                            {
                                "debug": inst.get("debug", 0),
                                "engine": inst["engine"],
                                "ins": [],
                                "outs": [],
                                "is_reset_sema": False,
                                "name": f"{inst['name']}-sw{k}",
                                "opcode": "Drain",
                                "sync_info": {"on_update": [], "on_wait": [w]},
                            }
                        )
                    si["on_wait"] = [waits[-1]]
                out.append(inst)
            bb["instructions"] = out
            first_block = False
    if not changed:
        return bir_json
    return json.dumps(bir).encode()


def _patched_compile_bir_kernel(bir_json, tmpdir, neff_name="file.neff"):
    return _ORIG_COMPILE_BIR(_split_multiwait_bir(bir_json), tmpdir, neff_name)


if getattr(concourse.bass2jax.compile_bir_kernel, "__name__", "") != (
    "_patched_compile_bir_kernel"
):
    concourse.bass2jax.compile_bir_kernel = _patched_compile_bir_kernel
    concourse.bass_utils.compile_bir_kernel = _patched_compile_bir_kernel

N_CORES = 8
N_FULL = 32     # full batch
NB = N_FULL // N_CORES  # examples per core
HG = 128        # H // P
WG = 128        # W // P
C = 64          # channels
P = 2           # patch size
DIM = C * P * P             # 256 floats per patch row
LFREE = WG * DIM            # floats per partition for one example
CB = 8                      # channels per store block
NCB = C // CB
SFREE = CB * P * 256
NJB = 8                     # load chunks per example
JB = WG // NJB


# jw extents of the load DMA instructions: small first chunks so the HWDGE
# doorbell (rung only after a whole instruction's descriptors are generated,
# ~4.6 ns/desc) fires early and the engines ramp fast.
LOAD_CHUNKS = (4, 12, 16, 16, 16, 16, 16, 16, 16)
assert sum(LOAD_CHUNKS) == WG


def _build_kernel(nc: bass.Bass, x: bass.AP, out: bass.AP):
    with tile.TileContext(nc) as tc:
        with (
            tc.tile_pool(name="lpool", bufs=1) as lpool,
            tc.tile_pool(name="spool", bufs=4) as spool,
        ):
            for n in range(NB):
                xv = x[n].rearrange("(jw ih) d -> ih jw d", ih=HG)
                ov = out[n].rearrange("c (ih pi) w -> ih c (pi w)", ih=HG)
                L = lpool.tile([128, LFREE], F32, tag="L")
                lv = L.rearrange("p (jw d) -> p jw d", d=DIM)
                j0 = 0
                for sz in LOAD_CHUNKS:
                    nc.sync.dma_start(
                        out=lv[:, j0 : j0 + sz, :], in_=xv[:, j0 : j0 + sz, :]
                    )
                    j0 += sz
                lshuf = L.rearrange(
                    "p (jw c pi pj) -> p jw c pi pj", jw=WG, c=C, pi=P, pj=P
                )

                def copy_block(cb, pi, pj, j0, j1, sv):
                    src = lshuf[:, j0:j1, cb * CB : (cb + 1) * CB, pi, pj]
                    src = src.transpose([0, 2, 1])  # [p, cl, jw-range]
                    dst = sv[:, :, pi, j0:j1, pj]   # [p, cl, jw-range]
                    nc.vector.tensor_copy(out=dst, in_=src)

                for cb in range(NCB):
                    S = spool.tile([128, SFREE], F32, tag="S")
                    sv = S.rearrange(
                        "p (cl pi jw pj) -> p cl pi jw pj", cl=CB, pi=P, jw=WG, pj=P
                    )
                    sfl = S.rearrange("p (cl piw) -> p cl piw", piw=P * 256)
                    if cb == 0:
                        # quarter-split jw: first three quarters' copies
                        # overlap the tail load DMAs, so after the load phase
                        # only one small copy batch remains before the first
                        # store can issue
                        JQ = WG // 4
                        for q in range(4):
                            for pi in range(P):
                                for pj in range(P):
                                    copy_block(cb, pi, pj, q * JQ, (q + 1) * JQ, sv)
                        # split the first store into 2-channel pieces: the
                        # HWDGE generates ~258 descriptors per piece, so the
                        # first store bytes hit HBM ~1.2us after dispatch
                        # instead of ~5us
                        for c2 in range(0, CB, 2):
                            nc.scalar.dma_start(
                                out=ov[:, c2 : c2 + 2, :],
                                in_=sfl[:, c2 : c2 + 2, :],
                            )
                    else:
                        for pi in range(P):
                            for pj in range(P):
                                copy_block(cb, pi, pj, 0, WG, sv)
                        nc.scalar.dma_start(
                            out=ov[:, cb * CB : (cb + 1) * CB, :], in_=sfl
                        )


_NC_CACHE = None


def _get_program() -> bass.Bass:
    global _NC_CACHE
    if _NC_CACHE is None:
        nc = bass.Bass("TRN2", num_devices=N_CORES)
        x = nc.dram_tensor("x", [NB, WG * HG, DIM], F32, kind="ExternalInput")
        out = nc.dram_tensor(
            "out", [NB, C, HG * P, WG * P], F32, kind="ExternalOutput"
        )
        _build_kernel(nc, x.ap(), out.ap())
        _NC_CACHE = nc
    return _NC_CACHE


def kernel(x: np.ndarray, H=256, W=256, **_unused) -> np.ndarray:
    x = np.ascontiguousarray(x, dtype=np.float32)
    assert x.shape == (N_FULL, WG * HG, DIM), x.shape
    nc = _get_program()
    shards = np.split(x, N_CORES, axis=0)
    in_maps = [{"x": s} for s in shards]
    res = run_bass_kernel_spmd(nc, in_maps, core_ids=list(range(N_CORES)))
    outs = [np.asarray(r["out"]) for r in res.results]
    return np.concatenate(outs, axis=0)


# revision 8
# speedup vs baseline: 1.1531x; 1.1346x over previous
"""DePatchEmbed Trainium2 kernel.

Full op: x (32, 16384, 256) f32 -> out (32, 64, 256, 256) f32 with
  out[n, c, 2*ih+pi, 2*jw+pj] = x[n, jw*128+ih, c*4+pi*2+pj]

Sharding: pure data-parallel over the batch dim — 4 examples per core on
8 NeuronCores. Per core the op is a local permutation done in one pass:

  load : x[n] -> L[ih; jw, d]  SBUF, partition = ih (1 KiB contiguous runs)
  DVE  : S[ih; cl, pi, w=2jw+pj] <- L[ih; jw, (c,pi,pj)]  (strided copies,
         data never leaves its partition)
  store: S -> out[n, c-block]  (2 KiB contiguous runs)
"""

import json

import numpy as np

import concourse.bass as bass
import concourse.bass_utils
import concourse.bass2jax
import concourse.mybir as mybir
from concourse import tile
from concourse.bass_utils import run_bass_kernel_spmd

F32 = mybir.dt.float32

# ---------------------------------------------------------------------------
# The bundled walrus accepts at most one sync-wait per instruction
# ("Too many sync wait commands" in CoreV3GenImpl::setupSyncWait), but Tile's
# kernel-tail Drain carries one wait per outstanding DMA-sem lane. Rewrite the
# BIR before compilation: split any instruction with N>1 waits into N-1
# single-wait Drains followed by the original instruction with one wait.
_ORIG_COMPILE_BIR = concourse.bass_utils.compile_bir_kernel


def _split_multiwait_bir(bir_json: bytes) -> bytes:
    bir = json.loads(bir_json)
    changed = False
    first_block = True
    for fn in bir.get("functions", []):
        for bb in fn.get("blocks", []):
            insts = bb.get("instructions", [])
            out = []
            for inst in insts:
                # Strip the entry const-pool barrier (this kernel reads no
                # const APs, so engines need not rendezvous before starting).
                if first_block and inst.get("opcode") in ("Drain", "EventSemaphore"):
                    si0 = inst.get("sync_info") or {}
                    sems = [
                        w.get("ant_name", "")
                        for w in si0.get("on_wait", []) + si0.get("on_update", [])
                    ]
                    if sems and all(s.startswith("barrier_") for s in sems):
                        changed = True
                        continue
                si = inst.get("sync_info")
                waits = si.get("on_wait", []) if si else []
                if len(waits) > 1:
                    changed = True
                    for k, w in enumerate(waits[:-1]):
                        out.append(
                            {
                                "debug": inst.get("debug", 0),
                                "engine": inst["engine"],
                                "ins": [],
                                "outs": [],
                                "is_reset_sema": False,
                                "name": f"{inst['name']}-sw{k}",
                                "opcode": "Drain",
                                "sync_info": {"on_update": [], "on_wait": [w]},
                            }
                        )
                    si["on_wait"] = [waits[-1]]
                out.append(inst)
            bb["instructions"] = out
            first_block = False
    if not changed:
        return bir_json
    return json.dumps(bir).encode()


def _patched_compile_bir_kernel(bir_json, tmpdir, neff_name="file.neff"):
    return _ORIG_COMPILE_BIR(_split_multiwait_bir(bir_json), tmpdir, neff_name)


if getattr(concourse.bass2jax.compile_bir_kernel, "__name__", "") != (
    "_patched_compile_bir_kernel"
):
    concourse.bass2jax.compile_bir_kernel = _patched_compile_bir_kernel
    concourse.bass_utils.compile_bir_kernel = _patched_compile_bir_kernel

N_CORES = 8
N_FULL = 32     # full batch
NB = N_FULL // N_CORES  # examples per core
HG = 128        # H // P
WG = 128        # W // P
C = 64          # channels
P = 2           # patch size
DIM = C * P * P             # 256 floats per patch row
LFREE = WG * DIM            # floats per partition for one example
CB = 8                      # channels per store block
NCB = C // CB
SFREE = CB * P * 256
NJB = 8                     # load chunks per example
JB = WG // NJB


# jw extents of the load DMA instructions: small first chunks so the HWDGE
# doorbell (rung only after a whole instruction's descriptors are generated,
# ~4.6 ns/desc) fires early and the engines ramp fast.
LOAD_CHUNKS = (4, 12, 16, 16, 16, 16, 16, 16, 16)
assert sum(LOAD_CHUNKS) == WG


def _build_kernel(nc: bass.Bass, x: bass.AP, out: bass.AP):
    with tile.TileContext(nc) as tc:
        with (
            tc.tile_pool(name="lpool", bufs=1) as lpool,
            tc.tile_pool(name="spool", bufs=4) as spool,
        ):
            for n in range(NB):
                xv = x[n].rearrange("(jw ih) d -> ih jw d", ih=HG)
                ov = out[n].rearrange("c (ih pi) w -> ih c (pi w)", ih=HG)
                L = lpool.tile([128, LFREE], F32, tag="L")
                lv = L.rearrange("p (jw d) -> p jw d", d=DIM)
                j0 = 0
                for sz in LOAD_CHUNKS:
                    nc.sync.dma_start(
                        out=lv[:, j0 : j0 + sz, :], in_=xv[:, j0 : j0 + sz, :]
                    )
                    j0 += sz
                lshuf = L.rearrange(
                    "p (jw c pi pj) -> p jw c pi pj", jw=WG, c=C, pi=P, pj=P
                )

                def copy_block(cb, pi, pj, j0, j1, sv):
                    src = lshuf[:, j0:j1, cb * CB : (cb + 1) * CB, pi, pj]
                    src = src.transpose([0, 2, 1])  # [p, cl, jw-range]
                    dst = sv[:, :, pi, j0:j1, pj]   # [p, cl, jw-range]
                    nc.vector.tensor_copy(out=dst, in_=src)

                for cb in range(NCB):
                    S = spool.tile([128, SFREE], F32, tag="S")
                    sv = S.rearrange(
                        "p (cl pi jw pj) -> p cl pi jw pj", cl=CB, pi=P, jw=WG, pj=P
                    )
                    sfl = S.rearrange("p (cl piw) -> p cl piw", piw=P * 256)
                    if cb == 0:
                        # quarter-split jw: first three quarters' copies
                        # overlap the tail load DMAs, so after the load phase
                        # only one small copy batch remains before the first
                        # store can issue
                        JQ = WG // 4
                        for q in range(4):
                            for pi in range(P):
                                for pj in range(P):
                                    copy_block(cb, pi, pj, q * JQ, (q + 1) * JQ, sv)
                    else:
                        for pi in range(P):
                            for pj in range(P):
                                copy_block(cb, pi, pj, 0, WG, sv)
                    # 2-channel store instructions: each generates fast
                    # (~256 descs) and walks HBM near-sequentially, which
                    # measures ~20% faster per descriptor than 8-channel
                    # c-strided stores
                    for c2 in range(0, CB, 2):
                        nc.scalar.dma_start(
                            out=ov[:, cb * CB + c2 : cb * CB + c2 + 2, :],
                            in_=sfl[:, c2 : c2 + 2, :],
                        )


_NC_CACHE = None


def _get_program() -> bass.Bass:
    global _NC_CACHE
    if _NC_CACHE is None:
        nc = bass.Bass("TRN2", num_devices=N_CORES)
        x = nc.dram_tensor("x", [NB, WG * HG, DIM], F32, kind="ExternalInput")
        out = nc.dram_tensor(
            "out", [NB, C, HG * P, WG * P], F32, kind="ExternalOutput"
        )
        _build_kernel(nc, x.ap(), out.ap())
        _NC_CACHE = nc
    return _NC_CACHE


def kernel(x: np.ndarray, H=256, W=256, **_unused) -> np.ndarray:
    x = np.ascontiguousarray(x, dtype=np.float32)
    assert x.shape == (N_FULL, WG * HG, DIM), x.shape
    nc = _get_program()
    shards = np.split(x, N_CORES, axis=0)
    in_maps = [{"x": s} for s in shards]
    res = run_bass_kernel_spmd(nc, in_maps, core_ids=list(range(N_CORES)))
    outs = [np.asarray(r["out"]) for r in res.results]
    return np.concatenate(outs, axis=0)


# revision 10
# speedup vs baseline: 1.1590x; 1.0051x over previous
"""DePatchEmbed Trainium2 kernel.

Full op: x (32, 16384, 256) f32 -> out (32, 64, 256, 256) f32 with
  out[n, c, 2*ih+pi, 2*jw+pj] = x[n, jw*128+ih, c*4+pi*2+pj]

Sharding: pure data-parallel over the batch dim — 4 examples per core on
8 NeuronCores. Per core the op is a local permutation done in one pass:

  load : x[n] -> L[ih; jw, d]  SBUF, partition = ih (1 KiB contiguous runs)
  DVE  : S[ih; cl, pi, w=2jw+pj] <- L[ih; jw, (c,pi,pj)]  (strided copies,
         data never leaves its partition)
  store: S -> out[n, c-block]  (2 KiB contiguous runs)
"""

import json

import numpy as np

import concourse.bass as bass
import concourse.bass_utils
import concourse.bass2jax
import concourse.mybir as mybir
from concourse import tile
from concourse.bass_utils import run_bass_kernel_spmd

F32 = mybir.dt.float32

# ---------------------------------------------------------------------------
# The bundled walrus accepts at most one sync-wait per instruction
# ("Too many sync wait commands" in CoreV3GenImpl::setupSyncWait), but Tile's
# kernel-tail Drain carries one wait per outstanding DMA-sem lane. Rewrite the
# BIR before compilation: split any instruction with N>1 waits into N-1
# single-wait Drains followed by the original instruction with one wait.
_ORIG_COMPILE_BIR = concourse.bass_utils.compile_bir_kernel


def _split_multiwait_bir(bir_json: bytes) -> bytes:
    bir = json.loads(bir_json)
    changed = False
    first_block = True
    for fn in bir.get("functions", []):
        for bb in fn.get("blocks", []):
            insts = bb.get("instructions", [])
            out = []
            for inst in insts:
                # Strip the entry const-pool barrier (this kernel reads no
                # const APs, so engines need not rendezvous before starting).
                if first_block and inst.get("opcode") in ("Drain", "EventSemaphore"):
                    si0 = inst.get("sync_info") or {}
                    sems = [
                        w.get("ant_name", "")
                        for w in si0.get("on_wait", []) + si0.get("on_update", [])
                    ]
                    if sems and all(s.startswith("barrier_") for s in sems):
                        changed = True
                        continue
                si = inst.get("sync_info")
                waits = si.get("on_wait", []) if si else []
                if len(waits) > 1:
                    changed = True
                    for k, w in enumerate(waits[:-1]):
                        out.append(
                            {
                                "debug": inst.get("debug", 0),
                                "engine": inst["engine"],
                                "ins": [],
                                "outs": [],
                                "is_reset_sema": False,
                                "name": f"{inst['name']}-sw{k}",
                                "opcode": "Drain",
                                "sync_info": {"on_update": [], "on_wait": [w]},
                            }
                        )
                    si["on_wait"] = [waits[-1]]
                out.append(inst)
            bb["instructions"] = out
            first_block = False
    if not changed:
        return bir_json
    return json.dumps(bir).encode()


def _patched_compile_bir_kernel(bir_json, tmpdir, neff_name="file.neff"):
    return _ORIG_COMPILE_BIR(_split_multiwait_bir(bir_json), tmpdir, neff_name)


if getattr(concourse.bass2jax.compile_bir_kernel, "__name__", "") != (
    "_patched_compile_bir_kernel"
):
    concourse.bass2jax.compile_bir_kernel = _patched_compile_bir_kernel
    concourse.bass_utils.compile_bir_kernel = _patched_compile_bir_kernel

N_CORES = 8
N_FULL = 32     # full batch
NB = N_FULL // N_CORES  # examples per core
HG = 128        # H // P
WG = 128        # W // P
C = 64          # channels
P = 2           # patch size
DIM = C * P * P             # 256 floats per patch row
LFREE = WG * DIM            # floats per partition for one example
CB = 8                      # channels per store block
NCB = C // CB
SFREE = CB * P * 256
NJB = 8                     # load chunks per example
JB = WG // NJB


# jw extents of the load DMA instructions: small first chunks so the HWDGE
# doorbell (rung only after a whole instruction's descriptors are generated,
# ~4.6 ns/desc) fires early and the engines ramp fast.
LOAD_CHUNKS = (4, 12, 16, 16, 16, 16, 16, 16, 8, 8)
assert sum(LOAD_CHUNKS) == WG

# cb0 copy batches, aligned to load-chunk boundaries; the last batches are
# tiny so the first store dispatches almost immediately after the load phase
CB0_SPLITS = (0, 32, 64, 96, 112, 120, 128)


def _build_kernel(nc: bass.Bass, x: bass.AP, out: bass.AP):
    with tile.TileContext(nc) as tc:
        with (
            tc.tile_pool(name="lpool", bufs=1) as lpool,
            tc.tile_pool(name="spool", bufs=4) as spool,
        ):
            for n in range(NB):
                xv = x[n].rearrange("(jw ih) d -> ih jw d", ih=HG)
                ov = out[n].rearrange("c (ih pi) w -> ih c (pi w)", ih=HG)
                L = lpool.tile([128, LFREE], F32, tag="L")
                lv = L.rearrange("p (jw d) -> p jw d", d=DIM)
                j0 = 0
                for sz in LOAD_CHUNKS:
                    nc.sync.dma_start(
                        out=lv[:, j0 : j0 + sz, :], in_=xv[:, j0 : j0 + sz, :]
                    )
                    j0 += sz
                lshuf = L.rearrange(
                    "p (jw c pi pj) -> p jw c pi pj", jw=WG, c=C, pi=P, pj=P
                )

                def copy_block(cb, pi, pj, j0, j1, sv):
                    src = lshuf[:, j0:j1, cb * CB : (cb + 1) * CB, pi, pj]
                    src = src.transpose([0, 2, 1])  # [p, cl, jw-range]
                    dst = sv[:, :, pi, j0:j1, pj]   # [p, cl, jw-range]
                    nc.vector.tensor_copy(out=dst, in_=src)

                for cb in range(NCB):
                    S = spool.tile([128, SFREE], F32, tag="S")
                    sv = S.rearrange(
                        "p (cl pi jw pj) -> p cl pi jw pj", cl=CB, pi=P, jw=WG, pj=P
                    )
                    sfl = S.rearrange("p (cl piw) -> p cl piw", piw=P * 256)
                    if cb == 0:
                        # jw batches aligned to load chunks: earlier batches
                        # overlap the tail load DMAs; the final tiny batch is
                        # all that separates load-end from the first store
                        for q in range(len(CB0_SPLITS) - 1):
                            for pi in range(P):
                                for pj in range(P):
                                    copy_block(
                                        cb, pi, pj,
                                        CB0_SPLITS[q], CB0_SPLITS[q + 1], sv,
                                    )
                    else:
                        for pi in range(P):
                            for pj in range(P):
                                copy_block(cb, pi, pj, 0, WG, sv)
                    # Narrow store instructions: generate fast (small desc
                    # count per doorbell) and walk HBM near-sequentially,
                    # ~20% faster per descriptor than 8-channel c-strided
                    # stores. The first two stores of each example are
                    # single-channel to open the store phase earliest.
                    if cb == 0:
                        widths = (1, 1, 2, 2, 2)
                    else:
                        widths = (2, 2, 2, 2)
                    c2 = 0
                    for wdt in widths:
                        nc.scalar.dma_start(
                            out=ov[:, cb * CB + c2 : cb * CB + c2 + wdt, :],
                            in_=sfl[:, c2 : c2 + wdt, :],
                        )
                        c2 += wdt


_NC_CACHE = None


def _get_program() -> bass.Bass:
    global _NC_CACHE
    if _NC_CACHE is None:
        nc = bass.Bass("TRN2", num_devices=N_CORES)
        x = nc.dram_tensor("x", [NB, WG * HG, DIM], F32, kind="ExternalInput")
        out = nc.dram_tensor(
            "out", [NB, C, HG * P, WG * P], F32, kind="ExternalOutput"
        )
        _build_kernel(nc, x.ap(), out.ap())
        _NC_CACHE = nc
    return _NC_CACHE


def kernel(x: np.ndarray, H=256, W=256, **_unused) -> np.ndarray:
    x = np.ascontiguousarray(x, dtype=np.float32)
    assert x.shape == (N_FULL, WG * HG, DIM), x.shape
    nc = _get_program()
    shards = np.split(x, N_CORES, axis=0)
    in_maps = [{"x": s} for s in shards]
    res = run_bass_kernel_spmd(nc, in_maps, core_ids=list(range(N_CORES)))
    outs = [np.asarray(r["out"]) for r in res.results]
    return np.concatenate(outs, axis=0)
